# revision 1
# baseline (speedup 1.0000x reference)
"""Trainium2 Bass kernel for nn_AttentionShareLocal — v5 (single core).

v2's exact per-window compute (4-bank S^T with row-group concurrency, proven
numerics) + two pipeline fixes:
  - software-pipelined emission: per step emit QK/exp/mul for window w and
    PV/normalize for window w-LAG, so the PE's in-order queue never waits
    inline on the ACT->DVE chain;
  - deep DMA prefetch: smaller groups (GRPQ=32 / GRPV=16) with 3-4 buffers,
    all DMAs on the otherwise-idle SP queue.
"""
import numpy as np
import ml_dtypes

import concourse.bass as bass
import concourse.tile as tile
from concourse import bacc, mybir
from concourse.bass_utils import run_bass_kernel_spmd

F32 = mybir.dt.float32
BF16 = mybir.dt.bfloat16

NCORES = 1
B, N, C = 2048, 49, 256
NH, D = 8, 32
GS = 7
WPC = B // NCORES          # windows per core = 256
GRPQ = 64                  # windows per q/k DMA group
GRPV = 32                  # windows per v/out DMA group
NG = WPC // GRPQ
NGV = WPC // GRPV
LAG = 3                    # software pipeline depth


def _build(wpc=WPC, num_devices=NCORES, repeat=1):
    ng = wpc // GRPQ
    ngv = wpc // GRPV
    nc = bacc.Bacc("TRN2", target_bir_lowering=False, debug=False,
                   num_devices=num_devices)
    qt_d = nc.declare_dram_parameter("qt", [ng * 2 * 128, GRPQ * N], BF16,
                                     isOutput=False)
    kt_d = nc.declare_dram_parameter("kt", [ng * 2 * 128, GRPQ * N], BF16,
                                     isOutput=False)
    va_d = nc.declare_dram_parameter("va", [ngv * N, GRPV * NH * 33], BF16,
                                     isOutput=False)
    expbT = nc.declare_dram_parameter("expbT", [N, NH * N], BF16,
                                      isOutput=False)
    out = nc.declare_dram_parameter("out", [ngv * N, GRPV * C], BF16,
                                    isOutput=True)

    qt_v = qt_d[:].rearrange("(g c p) x -> g c p x", c=2, p=128)
    kt_v = kt_d[:].rearrange("(g c p) x -> g c p x", c=2, p=128)
    va_v = va_d[:].rearrange("(g j) x -> g j x", j=N)
    out_v = out[:].rearrange("(g j) x -> g j x", j=N)

    with tile.TileContext(nc) as tc:
        with tc.tile_pool(name="const", bufs=1) as cpool, \
             tc.tile_pool(name="tsp", bufs=3) as tsp, \
             tc.tile_pool(name="et", bufs=4) as etp, \
             tc.tile_pool(name="io", bufs=2) as iop, \
             tc.tile_pool(name="sm", bufs=4) as smp, \
             tc.tile_pool(name="ps", bufs=1, space="PSUM") as ps, \
             tc.tile_pool(name="ps2", bufs=4, space="PSUM") as ps2:

            eb_sb = cpool.tile([N, NH * N], BF16)
            nc.sync.dma_start(eb_sb[:], expbT[:])

            qk_groups = {}
            vcur = {}
            ocur = {}

            def issue_qk(g):
                tiles = {}
                for nm, srcv in (("q", qt_v), ("k", kt_v)):
                    for cc in range(2):
                        t = tsp.tile([128, GRPQ * N], BF16,
                                     tag=f"{nm}t{cc}", name=f"{nm}{cc}")
                        nc.sync.dma_start(t[:], srcv[g, cc])
                        tiles[(nm, cc)] = t
                qk_groups[g] = tiles

            def issue_v(gv):
                vt = iop.tile([N, GRPV * NH * 33], BF16, tag="vpl",
                              name="vt")
                nc.sync.dma_start(vt[:], va_v[gv])
                vcur[gv] = vt

            def front(w):
                g, wq = divmod(w, GRPQ)
                qk = qk_groups[g]

                # QK^T into 4 PSUM banks (bank = h%4, matching the 4
                # concurrent PE row groups); chunks sequential per bank.
                sT = ps.tile([N, 4 * 512], F32, tag="sT")
                for h in range(NH):
                    ch, r = divmod(h, 4)
                    col = 512 * r + N * ch
                    nc.tensor.matmul(
                        sT[:, col:col + N],
                        qk[("k", ch)][32 * r:32 * r + 32, N * wq:N * wq + N],
                        qk[("q", ch)][32 * r:32 * r + 32, N * wq:N * wq + N],
                        start=True, stop=True,
                        tile_position=(32 * r, 0))

                sview = sT[:].rearrange("p (b c) -> p b c", b=4)[:, :, 0:2 * N]
                e0 = etp.tile([N, NH * N], BF16, tag="e0")
                nc.scalar.activation(
                    e0[:].rearrange("p (b c) -> p b c", b=4), sview,
                    mybir.ActivationFunctionType.Exp)
                eT = etp.tile([N, NH * N], BF16, tag="eT")
                nc.vector.tensor_mul(eT[:], e0[:], eb_sb[:])
                if wq == GRPQ - 1:
                    del qk_groups[g]
                return eT

            def back(w, eT):
                gv, wi = divmod(w, GRPV)
                if wi == 0:
                    ocur[gv] = iop.tile([N, GRPV * C], BF16, tag="o8",
                                        name="ot")
                o8 = ocur[gv]
                v4 = vcur[gv][:].rearrange("p (w h c) -> p w h c",
                                           w=GRPV, h=NH)
                oP = ps2.tile([N, NH * 33], F32, tag="oP")
                for h in range(NH):
                    ch, r = divmod(h, 4)
                    ecol = 2 * N * r + N * ch
                    nc.tensor.matmul(
                        oP[:, 33 * h:33 * (h + 1)],
                        eT[:, ecol:ecol + N],
                        v4[:, wi, h, :],
                        start=True, stop=True)
                ov = oP[:].rearrange("p (h c) -> p h c", h=NH)
                rt = smp.tile([N, NH], F32, tag="rt")
                nc.vector.reciprocal(rt[:], ov[:, :, 32])
                nc.vector.tensor_tensor(
                    o8[:, C * wi:C * (wi + 1)].rearrange(
                        "p (h c) -> p h c", h=NH),
                    ov[:, :, 0:32],
                    rt[:].unsqueeze(2).to_broadcast([N, NH, 32]),
                    mybir.AluOpType.mult)
                if wi == GRPV - 1:
                    nc.sync.dma_start(out_v[gv], o8[:])
                    del vcur[gv], ocur[gv]

            for _rep in range(repeat):
                pend = {}
                issue_qk(0)
                issue_v(0)
                for w in range(wpc + LAG):
                    if w < wpc:
                        nq = w + 32
                        if nq < wpc and nq % GRPQ == 0:
                            issue_qk(nq // GRPQ)
                        nv = w + 16
                        if nv < wpc and nv % GRPV == 0:
                            issue_v(nv // GRPV)
                        pend[w] = front(w)
                    if w >= LAG:
                        back(w - LAG, pend.pop(w - LAG))
    nc.compile()
    return nc


_CACHE = {}
TRACE = False
LAST_EXEC_NS = None


def _get_nc():
    if "nc" not in _CACHE:
        _CACHE["nc"] = _build()
    return _CACHE["nc"]


def _bias_table_host(W1, b1, W2, b2):
    r = np.arange(1 - GS, GS, dtype=np.float64)
    bh, bw = np.meshgrid(r, r, indexing="ij")
    biases = np.stack([bh.ravel(), bw.ravel()], axis=1)          # (169,2)
    pos = np.maximum(biases @ W1.astype(np.float64) + b1.astype(np.float64),
                     0.0) @ W2.astype(np.float64) + b2.astype(np.float64)
    coords = np.stack(np.meshgrid(np.arange(GS), np.arange(GS), indexing="ij"))
    cf = coords.reshape(2, -1)
    rel = (cf[:, :, None] - cf[:, None, :]).transpose(1, 2, 0).copy()
    rel[..., 0] += GS - 1
    rel[..., 1] += GS - 1
    rel[..., 0] *= 2 * GS - 1
    idx = rel.sum(-1)                                            # (49,49)
    return pos[idx].transpose(2, 0, 1)                           # (h,49,49)


def _prep_inputs(q, k, v, W1, b1, W2, b2):
    q = np.asarray(q, dtype=np.float32)
    k = np.asarray(k, dtype=np.float32)
    v = np.asarray(v, dtype=np.float32)

    bias = _bias_table_host(np.asarray(W1), np.asarray(b1),
                            np.asarray(W2), np.asarray(b2))      # (h,i,j)
    eb = np.exp(bias)
    # expbT[j, 98*(h%4) + 49*(h//4) + i] = exp(bias[h,i,j])  (bank-major)
    expbT = np.zeros((N, NH * N), np.float32)
    for h in range(NH):
        col = 98 * (h % 4) + 49 * (h // 4)
        expbT[:, col:col + N] = eb[h].T
    expbT = expbT.astype(ml_dtypes.bfloat16)

    scale = np.float32(D) ** np.float32(-0.5)
    qs = (q * scale).astype(ml_dtypes.bfloat16)
    kb = k.astype(ml_dtypes.bfloat16)
    qT = qs.reshape(NCORES, NG, GRPQ, N, 2, 128).transpose(0, 1, 4, 5, 2, 3)
    kT = kb.reshape(NCORES, NG, GRPQ, N, 2, 128).transpose(0, 1, 4, 5, 2, 3)
    qT = np.ascontiguousarray(qT).reshape(NCORES, NG * 2 * 128, GRPQ * N)
    kT = np.ascontiguousarray(kT).reshape(NCORES, NG * 2 * 128, GRPQ * N)
    va = np.ones((B, N, NH, 33), ml_dtypes.bfloat16)
    va[:, :, :, 0:32] = v.astype(ml_dtypes.bfloat16).reshape(B, N, NH, 32)
    va = va.reshape(NCORES, NGV, GRPV, N, NH * 33).transpose(0, 1, 3, 2, 4)
    va = np.ascontiguousarray(va).reshape(NCORES, NGV * N, GRPV * NH * 33)

    in_maps = []
    for c in range(NCORES):
        in_maps.append({
            "qt": qT[c],
            "kt": kT[c],
            "va": va[c],
            "expbT": expbT,
        })
    return in_maps


def _post(raw_outs):
    o = np.stack([np.asarray(r) for r in raw_outs])
    o = o.reshape(NCORES, NGV, N, GRPV, C).transpose(0, 1, 3, 2, 4)
    return np.ascontiguousarray(o).reshape(B, N, C).astype(np.float32)


def kernel(q, k, v, W1, b1, W2, b2, H=56, W=56):
    in_maps = _prep_inputs(q, k, v, W1, b1, W2, b2)
    nc = _get_nc()
    if TRACE:
        return _timed_run(nc, in_maps)
    res = run_bass_kernel_spmd(nc, in_maps, core_ids=list(range(NCORES)))
    return _post([res.results[c]["out"] for c in range(NCORES)])


def _timed_run(nc, in_maps, iters=50):
    import time
    import jax
    from jax.sharding import Mesh, PartitionSpec
    from jax.experimental.shard_map import shard_map
    from concourse import bass2jax as b2j
    from concourse import mybir as mb

    b2j.install_neuronx_cc_hook()
    in_names, out_names, out_avals, zero_outs = [], [], [], []
    pname = nc.partition_id_tensor.name if nc.partition_id_tensor else None
    for alloc in nc.m.functions[0].allocations:
        if not isinstance(alloc, mb.MemoryLocationSet):
            continue
        name = alloc.memorylocations[0].name
        if alloc.kind == "ExternalInput":
            if name != pname:
                in_names.append(name)
        elif alloc.kind == "ExternalOutput":
            out_names.append(name)
            shape = tuple(alloc.tensor_shape)
            dtype = mb.dt.np(alloc.dtype)
            out_avals.append(jax.core.ShapedArray(shape, dtype))
            zero_outs.append(np.zeros(shape, dtype))
    n_params = len(in_names)
    all_in_names = list(in_names) + list(out_names)
    if pname is not None:
        all_in_names.append(pname)

    def _body(*args):
        operands = list(args)
        if pname is not None:
            operands.append(b2j.partition_id_tensor())
        return tuple(b2j._bass_exec_p.bind(
            *operands,
            out_avals=tuple(out_avals),
            in_names=tuple(all_in_names),
            out_names=tuple(out_names),
            lowering_input_output_aliases=(),
            sim_require_finite=True,
            sim_require_nnan=True,
            nc=nc,
        ))

    devices = jax.devices()[:NCORES]
    mesh = Mesh(np.asarray(devices), ("core",))
    nin = n_params + len(zero_outs)
    sharded = jax.jit(shard_map(
        _body, mesh=mesh, in_specs=(PartitionSpec("core"),) * nin,
        out_specs=(PartitionSpec("core"),) * len(out_names), check_rep=False),
        keep_unused=True)

    concat_in = [np.concatenate([np.asarray(in_maps[c][nm])
                                 for c in range(NCORES)], axis=0)
                 for nm in in_names]
    concat_zeros = [np.zeros((NCORES * z.shape[0], *z.shape[1:]), z.dtype)
                    for z in zero_outs]
    dev_in = [jax.device_put(a) for a in concat_in + concat_zeros]

    out = sharded(*dev_in)
    jax.block_until_ready(out)

    @jax.jit
    def triv(x):
        return x * 2.0
    small = jax.device_put(np.zeros((NCORES * 8,), np.float32),
                           jax.sharding.NamedSharding(mesh, PartitionSpec("core")))
    jax.block_until_ready(triv(small))

    # Alternate dispatch-baseline and kernel loops over several rounds and
    # take the best-paired difference: the RPC dispatch overhead drifts with
    # ambient load, so a single round is +-1 ms noisy.
    diffs = []
    for rnd in range(5):
        o2 = small
        t0 = time.time()
        for _ in range(iters):
            o2 = triv(o2)
        jax.block_until_ready(o2)
        t_base = (time.time() - t0) / iters
        t0 = time.time()
        for _ in range(iters):
            out = sharded(*dev_in)
        jax.block_until_ready(out)
        t_kernel = (time.time() - t0) / iters
        diffs.append(t_kernel - t_base)
        print(f"round {rnd}: kernel {t_kernel*1e6:.1f} us/iter, "
              f"dispatch baseline {t_base*1e6:.1f} us/iter, "
              f"diff {(t_kernel-t_base)*1e6:.1f} us")

    global LAST_EXEC_NS
    LAST_EXEC_NS = int(max(0.0, min(diffs)) * 1e9)

    res = [np.asarray(out[0]).reshape(NCORES, *out_avals[0].shape)[c]
           for c in range(NCORES)]
    return _post(res)

